# revision 34
# baseline (speedup 1.0000x reference)
"""Trainium2 Bass kernel v7 for nn_BTokenizer (residual MLP tokenizer block).

Computes, for x [16384, 1024]:
    y = x @ Win.T + bin
    6x: y = y + gelu(LN(y) @ Wb[i].T + bb[i])
    out = y @ Wout.T + bout          -> [16384, 2048]

Data-parallel over tokens (2048/core). FEATURE-MAJOR resident state: y is
kept as y.T [feature(part), token(free)] in bf16; weights are the stationary
matmul operand; no transposes. LayerNorm mean-subtraction is folded into the
weights on the host (W' = W - outer(rowsum(W), ones)/D), so the normalize is
a single multiply by rstd.

v10 (over v6): the stats pipeline is rebalanced so the PE never stalls on
the DVE/Pool chain, and the Act engine runs gelu/identity/square ONLY (one
activation table, zero ACT_TABLE_LOADs in steady state):
  - rstd = rsqrt(var+eps) via Newton iteration on DVE (custom RSQRT_NR op,
    one instruction per iteration; iter 1 is affine in var using a
    per-layer seed constant estimated on the host from a token subsample;
    the final iter writes the bf16 rstd slice directly). This removes the
    Act Sqrt and its ~1.3us table swaps entirely;
  - y^2 squares on Act for pairs f=1,3,5 (square shares every Act table)
    and on DVE for the critical last pair f=7; qa pair-adds on DVE;
    sa pair-adds on Pool;
  - no lvl2/3 reduction trees: the partition-reduce matmuls accumulate the
    four raw pair-partials directly in PSUM (4-step start/stop groups) with
    1/D weights, so PSUM holds mu and E[y^2]; var = (E[y^2] + eps) - mu^2
    is one fused DVE op;
  - the deferred finish of the previous half's stats is spread over the
    next half's f=3..6 slots: chunk-0 reduce at f==3 (psS), chunk-1 reduce
    at f==4 (borrows a psG tile so chunk 0's bank WAR causes no PE
    bubble) + chunk-0 rstd, chunk-1 rstd + norm kt0-3 at f==5, norm kt4-7
    at f==6.
Measured (repeat-delta convention): ~546-551us unthrottled; the chip's
power brake (gpio throttle) adds 0-90us run-to-run regardless of kernel.
PSUM: psG 3 bufs (6 banks) + psS 1 buf (2 banks).
"""

import contextlib

import numpy as np

import concourse.bass as bass
import concourse.tile as tile
from concourse import bacc, mybir
from concourse import bass_utils
from concourse import dve_ops

F32 = mybir.dt.float32
BF16 = mybir.dt.bfloat16
AF = mybir.ActivationFunctionType
OP = mybir.AluOpType


def _register_op(name, spec_fn):
    """Register a custom DVE op at import (monkeypatching dve_ops.OPS), with
    the uops sha computed on the fly."""
    if hasattr(dve_ops, name):
        return getattr(dve_ops, name)
    from concourse.dve_spec import lower
    from concourse.dve_uop import DveOpSpec

    spec = spec_fn()
    row = dve_ops._CUSTOM_DVE_ROW_BASE + len(dve_ops.OPS)
    shas = {}
    for ver in ("v3", "v4"):
        try:
            uops = lower(spec, ver=ver)
            s = DveOpSpec(name=name, opcode=row, uops=uops, rd1_en=True)
            shas[ver] = s.sha(ver)
        except Exception:
            pass
    op = dve_ops.DveOp(name, spec, subdim=False, uops_sha=shas)
    dve_ops.OPS.append(op)
    dve_ops._SUB_OPCODE_FOR_NAME[name] = row
    setattr(dve_ops, name, op)
    return op


def _rsqrt_spec():
    """One Newton-Raphson rsqrt step r' = r*(s0 - s1*v*r^2), on DVE - so no
    Act-table (Sqrt) loads are needed anywhere in the kernel."""
    from concourse.dve_spec import Spec, Src0, Src1, C0, C1, sq

    return Spec(
        body=Src0 * (C0 - C1 * (Src1 * sq(Src0))),
        reference=lambda in0, in1, c0, c1, c2: in0 * (c0 - c1 * (in1 * in0 * in0)),
    )


def _sqsq_spec():
    """qa pair partial a^2 + b^2 in one DVE op (replaces two Act/DVE squares
    plus an add)."""
    from concourse.dve_spec import Spec, Src0, Src1, sq

    return Spec(
        body=sq(Src0) + sq(Src1),
        reference=lambda in0, in1, c0, c1, c2: in0 * in0 + in1 * in1,
    )


RSQRT_NR = _register_op("RSQRT_NR_ANT", _rsqrt_spec)
SQSQ = _register_op("SQSQ_ANT", _sqsq_spec)

N_CORES = 8
N_TOKENS = 16384
T = N_TOKENS // N_CORES  # 2048 tokens per core
D = 1024                 # in/hidden width
NOUT = 2048
NB = 6                   # inner residual blocks
EPS = 1e-5
KT = D // 128            # 8 k-tiles (feature partitions)
FT = D // 128            # 8 fout-tiles for hidden layers
FT3 = NOUT // 128        # 16 fout-tiles for the output layer
NC4 = T // 512           # 4 psum-bank token chunks
NC2 = T // 1024          # 2 token halves


def build_nc(repeat: int = 1, seeds: tuple = (1.2,) * 6):
    nc = bacc.Bacc("TRN2", target_bir_lowering=False, debug=False, num_devices=1)

    xt = nc.dram_tensor("xt", [D, T], BF16, kind="ExternalInput").ap()
    wint = nc.dram_tensor("wint", [D, D], BF16, kind="ExternalInput").ap()
    wbt = nc.dram_tensor("wbt", [NB, D, D], BF16, kind="ExternalInput").ap()
    woutt = nc.dram_tensor("woutt", [D, NOUT], BF16, kind="ExternalInput").ap()
    bin_b = nc.dram_tensor("bin_b", [D], F32, kind="ExternalInput").ap()
    bb = nc.dram_tensor("bb", [NB, D], F32, kind="ExternalInput").ap()
    bout = nc.dram_tensor("bout", [NOUT], F32, kind="ExternalInput").ap()
    out_t = nc.dram_tensor("out_t", [NOUT, T], F32, kind="ExternalOutput").ap()

    with tile.TileContext(nc) as tc:
        with contextlib.ExitStack() as ctx:
            kernel_body(ctx, tc, xt, wint, wbt, woutt, bin_b, bb, bout, out_t,
                        repeat, seeds)
    nc.finalize()
    return nc


def kernel_body(ctx, tc, xt, wint, wbt, woutt, bin_b, bb, bout, out_t, repeat,
                seeds):
    nc = tc.nc

    singles = ctx.enter_context(tc.tile_pool(name="singles", bufs=1))
    ypool = ctx.enter_context(tc.tile_pool(name="ypool", bufs=1))
    tpool = ctx.enter_context(tc.tile_pool(name="tpool", bufs=1))
    wpool = ctx.enter_context(tc.tile_pool(name="wpool", bufs=2))
    gpool = ctx.enter_context(tc.tile_pool(name="gpool", bufs=4))
    ysqpool = ctx.enter_context(tc.tile_pool(name="ysqpool", bufs=2))
    statpool = ctx.enter_context(tc.tile_pool(name="statpool", bufs=2))
    rstdpool = ctx.enter_context(tc.tile_pool(name="rstdpool", bufs=2))
    ostpool = ctx.enter_context(tc.tile_pool(name="ostpool", bufs=3))
    psG = ctx.enter_context(tc.tile_pool(name="psG", bufs=3, space="PSUM"))
    psS = ctx.enter_context(tc.tile_pool(name="psS", bufs=1, space="PSUM"))

    # constants / biases (per-partition layouts)
    invD = singles.tile([128, 128], BF16)
    nc.vector.memset(invD, 1.0 / D)  # 2^-10, exact in bf16
    bin_t = singles.tile([128, FT], F32)
    nc.sync.dma_start(bin_t, bin_b.rearrange("(f p) -> p f", p=128))
    bb_t = singles.tile([128, NB, FT], F32)
    nc.sync.dma_start(bb_t, bb.rearrange("i (f p) -> p i f", p=128))
    bout_t = singles.tile([128, FT3], F32)
    nc.sync.dma_start(bout_t, bout.rearrange("(f p) -> p f", p=128))

    # resident state: y.T and t = x.T / normalized y.T, feature-major bf16
    y_t = ypool.tile([128, KT, T], BF16)
    t_t = tpool.tile([128, KT, T], BF16)

    # ---- stats helpers (per token half: 2 chunks of 512) ----
    def make_stats(half):
        sa = [[ysqpool.tile([128, 512], BF16, name="sp", tag=f"sp{j}_{c}")
               for c in range(2)] for j in range(4)]
        qa = [[ysqpool.tile([128, 512], BF16, name="qp", tag=f"qp{j}_{c}")
               for c in range(2)] for j in range(4)]
        return half, sa, qa

    def stats_pair(stats, f):
        """Level-1 kt-partials for feature pair (f-1, f) on this half's two
        chunks: sa = y+y' on Pool; qa = y^2+y'^2 in one custom DVE op."""
        half, sa, qa = stats
        j = f // 2
        for ci in range(2):
            cs = bass.ts(2 * half + ci, 512)
            nc.gpsimd.tensor_tensor(sa[j][ci], y_t[:, f - 1, cs],
                                    y_t[:, f, cs], OP.add)
            nc.vector._custom_dve(SQSQ, out=qa[j][ci],
                                  in0=y_t[:, f - 1, cs], in1=y_t[:, f, cs])

    def stats_reduce_var(stats, ci, sq):
        """Chunk ci: partition-reduce with 4-step PSUM accumulation over the
        raw pair partials (PE, 1/D weights -> mu, E[y^2]; no DVE lvl2/3
        trees), musq on Act, var = (qd + eps) - musq on DVE."""
        half, sa, qa = stats
        mu, qd = sq[:, 0, :], sq[:, 1, :]
        for j in (0, 2):
            nc.tensor.matmul(mu, invD, sa[j][ci], start=(j == 0),
                             stop=(j == 2))
        for j in (0, 2):
            nc.tensor.matmul(qd, invD, qa[j][ci], start=(j == 0),
                             stop=(j == 2))
        musq = statpool.tile([128, 512], F32, name="musq", tag=f"musq{ci}")
        nc.scalar.square(musq, mu)
        var = statpool.tile([128, 512], F32, name="var", tag=f"var{ci}")
        nc.vector.scalar_tensor_tensor(var, qd, EPS, musq,
                                       OP.add, OP.subtract)
        return var

    def stats_rstd(var, half, ci, rstd, seed):
        """Chunk ci: rstd = rsqrt(var) by Newton iteration, entirely on
        DVE (no Act Sqrt -> no ACT_TABLE_LOADs). Iter 1 is affine in var
        (seed is a per-layer constant); the last iter writes bf16 rstd."""
        c = 2 * half + ci
        r1 = statpool.tile([128, 512], F32, name="rs", tag=f"rs{ci}a")
        nc.vector.tensor_scalar(r1, var, -0.5 * seed**3, 1.5 * seed,
                                OP.mult, OP.add)
        r2 = statpool.tile([128, 512], F32, name="rs", tag=f"rs{ci}b")
        nc.vector._custom_dve(RSQRT_NR, out=r2, in0=r1, in1=var,
                              s0=1.5, s1=0.5)
        nc.vector._custom_dve(RSQRT_NR, out=rstd[:, bass.ts(c, 512)],
                              in0=r2, in1=var, s0=1.5, s1=0.5)

    def norm_part(rstd, half, k0, k1):
        """t = y * rstd for kt in [k0, k1) on this half."""
        hs = bass.ts(half, 1024)
        for kt in range(k0, k1):
            nc.vector.tensor_tensor(t_t[:, kt, hs], y_t[:, kt, hs],
                                    rstd[:, hs], OP.mult)

    def pending_step(pending, f, varbox):
        """Spread the deferred finish of the previous half's stats over this
        half's f slots. Returns pending (None once fully consumed)."""
        if pending is None:
            return None
        st, rstd, half, seed = pending
        if f == 2:
            # lvl2 combines (4 partials -> 2) on DVE, emitted here so the
            # waits on Pool's last sa partials never head-block the DVE
            # queue at the half boundary.
            _, sa, qa = st
            for ci in range(2):
                nc.vector.tensor_tensor(sa[0][ci], sa[0][ci], sa[1][ci],
                                        OP.add)
                nc.vector.tensor_tensor(sa[2][ci], sa[2][ci], sa[3][ci],
                                        OP.add)
                nc.vector.tensor_tensor(qa[0][ci], qa[0][ci], qa[1][ci],
                                        OP.add)
                nc.vector.tensor_tensor(qa[2][ci], qa[2][ci], qa[3][ci],
                                        OP.add)
        elif f == 3:
            sq0 = psS.tile([128, 2, 512], F32, name="SQ", tag="SQ")
            varbox.append(stats_reduce_var(st, 0, sq0))
        elif f == 4:
            sq1 = psG.tile([128, 2, 512], F32, name="G", tag="G")
            varbox.append(stats_reduce_var(st, 1, sq1))
            stats_rstd(varbox.pop(0), half, 0, rstd, seed)
        elif f == 5:
            stats_rstd(varbox.pop(0), half, 1, rstd, seed)
            norm_part(rstd, half, 0, 4)
        elif f == 6:
            norm_part(rstd, half, 4, 8)
            return None
        return pending

    def mm_half(w_tile, f, half, rhs):
        """8-kt accumulation for fout tile f on this half's two chunks."""
        G = psG.tile([128, 2, 512], F32, name="G", tag="G")
        for kt in range(KT):
            lhsT = w_tile[:, kt, bass.ts(f, 128)]
            for ci in range(2):
                nc.tensor.matmul(G[:, ci, :], lhsT,
                                 rhs[:, kt, bass.ts(2 * half + ci, 512)],
                                 start=(kt == 0), stop=(kt == KT - 1))
        return G

    for _rep in range(repeat):
        # ---------------- Phase 1: y.T = (x @ Win.T + bin).T ----------------
        w_in = wpool.tile([128, KT, D], BF16, tag="w")
        nc.sync.dma_start(w_in, wint.rearrange("(kt p) n -> p kt n", p=128))
        for kt in range(KT):
            nc.sync.dma_start(t_t[:, kt, :],
                              xt.rearrange("(kt p) t -> p kt t", p=128)[:, kt, :])

        rstd_next = rstdpool.tile([128, T], BF16, name="rstd", tag="rstd")
        pending = None  # (stats, rstd, half) whose finish+norm is deferred
        varbox = []
        for half in range(NC2):
            st = make_stats(half)
            for f in range(FT):
                G = mm_half(w_in, f, half, t_t)
                nc.scalar.activation(
                    y_t[:, f, bass.ts(half, 1024)],
                    G.rearrange("p a b -> p (a b)"),
                    AF.Identity, bias=bin_t[:, bass.ds(f, 1)])
                if f % 2 == 1:
                    stats_pair(st, f)
                pending = pending_step(pending, f, varbox)
            pending = (st, rstd_next, half, seeds[0])

        # ---------------- Phase 2: residual blocks ----------------
        for i in range(NB):
            wb = wpool.tile([128, KT, D], BF16, tag="w")
            nc.sync.dma_start(wb, wbt[i].rearrange("(kt p) n -> p kt n", p=128))
            if i < NB - 1:
                rstd_next = rstdpool.tile([128, T], BF16, name="rstd", tag="rstd")
            for half in range(NC2):
                st = make_stats(half) if i < NB - 1 else None
                for f in range(FT):
                    G = mm_half(wb, f, half, t_t)
                    g = gpool.tile([128, 1024], BF16, name="g", tag="g")
                    nc.scalar.activation(
                        g, G.rearrange("p a b -> p (a b)"),
                        AF.Gelu, bias=bb_t[:, i, bass.ds(f, 1)])
                    nc.vector.tensor_tensor(y_t[:, f, bass.ts(half, 1024)],
                                            y_t[:, f, bass.ts(half, 1024)],
                                            g, OP.add)
                    if st is not None and f % 2 == 1:
                        stats_pair(st, f)
                    pending = pending_step(pending, f, varbox)
                if st is not None:
                    pending = (st, rstd_next, half, seeds[1 + i])

        # ---------------- Phase 3: out.T = (y @ Wout.T + bout).T ------------
        w3a = wpool.tile([128, KT, D], BF16, tag="w")
        nc.sync.dma_start(w3a, woutt[:, 0:D].rearrange("(kt p) n -> p kt n", p=128))
        w3b = wpool.tile([128, KT, D], BF16, tag="w")
        nc.sync.dma_start(w3b, woutt[:, D:NOUT].rearrange("(kt p) n -> p kt n", p=128))
        for half_w, w3 in ((0, w3a), (1, w3b)):
            for f in range(FT):
                fg = half_w * FT + f
                ost = ostpool.tile([128, T], F32, name="ost", tag="ost")
                for half in range(NC2):
                    G = mm_half(w3, f, half, y_t)
                    nc.scalar.activation(
                        ost[:, bass.ts(half, 1024)],
                        G.rearrange("p a b -> p (a b)"),
                        AF.Identity, bias=bout_t[:, bass.ds(fg, 1)])
                nc.gpsimd.dma_start(out_t[bass.ts(fg, 128), :], ost)


_CACHED_NC = {}


def _estimate_seeds(x, Win, bin_b, Wb, bb):
    """Per-stats-layer rsqrt(mean token variance) — Newton seeds for the
    on-chip rstd iteration. Estimated from a strided token subsample with
    the same LN/gelu math as the reference (seed needs only ~20% accuracy;
    4 Newton iterations converge from up to ~50% off)."""
    try:
        from scipy.special import erf

        def gelu(h):
            return 0.5 * h * (1.0 + erf(h / np.sqrt(2.0)))
    except ImportError:
        def gelu(h):
            return 0.5 * h * (1.0 + np.tanh(0.7978845608 * (h + 0.044715 * h**3)))

    xs = np.asarray(x[:: max(1, x.shape[0] // 512)], np.float32)
    Win = np.asarray(Win, np.float32)
    Wb = np.asarray(Wb, np.float32)
    bb = np.asarray(bb, np.float32)
    y = xs @ Win.T + np.asarray(bin_b, np.float32)
    seeds = []
    for i in range(NB):
        var = y.var(axis=1) + EPS
        seeds.append(float(1.0 / np.sqrt(var.mean())))
        mu = y.mean(axis=1, keepdims=True)
        t = (y - mu) / np.sqrt(var[:, None])
        h = t @ Wb[i].T + bb[i]
        y = y + gelu(h)
    return tuple(round(s, 4) for s in seeds)


def _prep_inputs(x, Win, bin_b, Wb, bb, Wout, bout_b):
    import ml_dtypes
    x = np.asarray(x, np.float32)
    Win = np.asarray(Win, np.float32)
    Wb = np.asarray(Wb, np.float32)
    Wout = np.asarray(Wout, np.float32)
    # fold LN mean-subtraction into the inner-block weights:
    # W' = W - outer(rowsum(W), ones)/D  so  W' @ (y*rstd) == ((y-mu)*rstd) @ W.T
    Wbp = Wb - Wb.sum(axis=2, keepdims=True) / D
    xt = np.ascontiguousarray(x.T).astype(ml_dtypes.bfloat16)
    wint = np.ascontiguousarray(Win.T).astype(ml_dtypes.bfloat16)
    wbt = np.ascontiguousarray(Wbp.transpose(0, 2, 1)).astype(ml_dtypes.bfloat16)
    woutt = np.ascontiguousarray(Wout.T).astype(ml_dtypes.bfloat16)
    return (xt, wint, wbt, woutt, np.asarray(bin_b, np.float32),
            np.asarray(bb, np.float32), np.asarray(bout_b, np.float32))


def make_in_maps(x, Win, bin_b, Wb, bb, Wout, bout_b):
    xt, wint, wbt, woutt, bin_arr, bb_arr, bout_arr = _prep_inputs(
        x, Win, bin_b, Wb, bb, Wout, bout_b)
    in_maps = []
    for c in range(N_CORES):
        in_maps.append({
            "xt": np.ascontiguousarray(xt[:, c * T:(c + 1) * T]),
            "wint": wint, "wbt": wbt, "woutt": woutt,
            "bin_b": bin_arr, "bb": bb_arr, "bout": bout_arr,
        })
    return in_maps


def kernel(x, Win, bin_b, Wb, bb, Wout, bout_b):
    seeds = _estimate_seeds(x, Win, bin_b, Wb, bb)
    nc = _CACHED_NC.get(seeds)
    if nc is None:
        nc = _CACHED_NC[seeds] = build_nc(seeds=seeds)
    in_maps = make_in_maps(x, Win, bin_b, Wb, bb, Wout, bout_b)
    res = bass_utils.run_bass_kernel_spmd(nc, in_maps, list(range(N_CORES)))
    return np.concatenate(
        [np.ascontiguousarray(res.results[c]["out_t"].T) for c in range(N_CORES)],
        axis=0)


# revision 36
# speedup vs baseline: 1.0348x; 1.0348x over previous
"""Trainium2 Bass kernel v7 for nn_BTokenizer (residual MLP tokenizer block).

Computes, for x [16384, 1024]:
    y = x @ Win.T + bin
    6x: y = y + gelu(LN(y) @ Wb[i].T + bb[i])
    out = y @ Wout.T + bout          -> [16384, 2048]

Data-parallel over tokens (2048/core). FEATURE-MAJOR resident state: y is
kept as y.T [feature(part), token(free)] in bf16; weights are the stationary
matmul operand; no transposes. LayerNorm mean-subtraction is folded into the
weights on the host (W' = W - outer(rowsum(W), ones)/D), so the normalize is
a single multiply by rstd.

v10 (over v6): the stats pipeline is rebalanced so the PE never stalls on
the DVE/Pool chain, and the Act engine runs gelu/identity/square ONLY (one
activation table, zero ACT_TABLE_LOADs in steady state):
  - rstd = rsqrt(var+eps) via Newton iteration on DVE (custom RSQRT_NR op,
    one instruction per iteration; iter 1 is affine in var using a
    per-layer seed constant estimated on the host from a token subsample;
    the final iter writes the bf16 rstd slice directly). This removes the
    Act Sqrt and its ~1.3us table swaps entirely;
  - y^2 squares on Act for pairs f=1,3,5 (square shares every Act table)
    and on DVE for the critical last pair f=7; qa pair-adds on DVE;
    sa pair-adds on Pool;
  - no lvl2/3 reduction trees: the partition-reduce matmuls accumulate the
    four raw pair-partials directly in PSUM (4-step start/stop groups) with
    1/D weights, so PSUM holds mu and E[y^2]; var = (E[y^2] + eps) - mu^2
    is one fused DVE op;
  - the deferred finish of the previous half's stats is spread over the
    next half's f=3..6 slots: chunk-0 reduce at f==3 (psS), chunk-1 reduce
    at f==4 (borrows a psG tile so chunk 0's bank WAR causes no PE
    bubble) + chunk-0 rstd, chunk-1 rstd + norm kt0-3 at f==5, norm kt4-7
    at f==6.
Measured (repeat-delta convention): ~546-551us unthrottled; the chip's
power brake (gpio throttle) adds 0-90us run-to-run regardless of kernel.
PSUM: psG 3 bufs (6 banks) + psS 1 buf (2 banks).
"""

import contextlib

import numpy as np

import concourse.bass as bass
import concourse.tile as tile
from concourse import bacc, mybir
from concourse import bass_utils
from concourse import dve_ops

F32 = mybir.dt.float32
BF16 = mybir.dt.bfloat16
AF = mybir.ActivationFunctionType
OP = mybir.AluOpType


def _register_op(name, spec_fn):
    """Register a custom DVE op at import (monkeypatching dve_ops.OPS), with
    the uops sha computed on the fly."""
    if hasattr(dve_ops, name):
        return getattr(dve_ops, name)
    from concourse.dve_spec import lower
    from concourse.dve_uop import DveOpSpec

    spec = spec_fn()
    row = dve_ops._CUSTOM_DVE_ROW_BASE + len(dve_ops.OPS)
    shas = {}
    for ver in ("v3", "v4"):
        try:
            uops = lower(spec, ver=ver)
            s = DveOpSpec(name=name, opcode=row, uops=uops, rd1_en=True)
            shas[ver] = s.sha(ver)
        except Exception:
            pass
    op = dve_ops.DveOp(name, spec, subdim=False, uops_sha=shas)
    dve_ops.OPS.append(op)
    dve_ops._SUB_OPCODE_FOR_NAME[name] = row
    setattr(dve_ops, name, op)
    return op


def _rsqrt_spec():
    """One Newton-Raphson rsqrt step r' = r*(s0 - s1*v*r^2), on DVE - so no
    Act-table (Sqrt) loads are needed anywhere in the kernel."""
    from concourse.dve_spec import Spec, Src0, Src1, C0, C1, sq

    return Spec(
        body=Src0 * (C0 - C1 * (Src1 * sq(Src0))),
        reference=lambda in0, in1, c0, c1, c2: in0 * (c0 - c1 * (in1 * in0 * in0)),
    )


def _sqsq_spec():
    """qa pair partial a^2 + b^2 in one DVE op (replaces two Act/DVE squares
    plus an add)."""
    from concourse.dve_spec import Spec, Src0, Src1, sq

    return Spec(
        body=sq(Src0) + sq(Src1),
        reference=lambda in0, in1, c0, c1, c2: in0 * in0 + in1 * in1,
    )


RSQRT_NR = _register_op("RSQRT_NR_ANT", _rsqrt_spec)
SQSQ = _register_op("SQSQ_ANT", _sqsq_spec)

N_CORES = 8
N_TOKENS = 16384
T = N_TOKENS // N_CORES  # 2048 tokens per core
D = 1024                 # in/hidden width
NOUT = 2048
NB = 6                   # inner residual blocks
EPS = 1e-5
KT = D // 128            # 8 k-tiles (feature partitions)
FT = D // 128            # 8 fout-tiles for hidden layers
FT3 = NOUT // 128        # 16 fout-tiles for the output layer
NC4 = T // 512           # 4 psum-bank token chunks
NC2 = T // 1024          # 2 token halves


def build_nc(repeat: int = 1, seeds: tuple = (1.2,) * 6):
    nc = bacc.Bacc("TRN2", target_bir_lowering=False, debug=False, num_devices=1)

    xt = nc.dram_tensor("xt", [D, T], BF16, kind="ExternalInput").ap()
    wint = nc.dram_tensor("wint", [D, D], BF16, kind="ExternalInput").ap()
    wbt = nc.dram_tensor("wbt", [NB, D, D], BF16, kind="ExternalInput").ap()
    woutt = nc.dram_tensor("woutt", [D, NOUT], BF16, kind="ExternalInput").ap()
    bin_b = nc.dram_tensor("bin_b", [D], F32, kind="ExternalInput").ap()
    bb = nc.dram_tensor("bb", [NB, D], F32, kind="ExternalInput").ap()
    bout = nc.dram_tensor("bout", [NOUT], F32, kind="ExternalInput").ap()
    out_t = nc.dram_tensor("out_t", [NOUT, T], F32, kind="ExternalOutput").ap()

    with tile.TileContext(nc) as tc:
        with contextlib.ExitStack() as ctx:
            kernel_body(ctx, tc, xt, wint, wbt, woutt, bin_b, bb, bout, out_t,
                        repeat, seeds)
    nc.finalize()
    return nc


def kernel_body(ctx, tc, xt, wint, wbt, woutt, bin_b, bb, bout, out_t, repeat,
                seeds):
    nc = tc.nc

    singles = ctx.enter_context(tc.tile_pool(name="singles", bufs=1))
    ypool = ctx.enter_context(tc.tile_pool(name="ypool", bufs=1))
    tpool = ctx.enter_context(tc.tile_pool(name="tpool", bufs=1))
    wpool = ctx.enter_context(tc.tile_pool(name="wpool", bufs=2))
    gpool = ctx.enter_context(tc.tile_pool(name="gpool", bufs=4))
    ysqpool = ctx.enter_context(tc.tile_pool(name="ysqpool", bufs=2))
    statpool = ctx.enter_context(tc.tile_pool(name="statpool", bufs=2))
    rstdpool = ctx.enter_context(tc.tile_pool(name="rstdpool", bufs=2))
    ostpool = ctx.enter_context(tc.tile_pool(name="ostpool", bufs=3))
    psG = ctx.enter_context(tc.tile_pool(name="psG", bufs=3, space="PSUM"))
    psS = ctx.enter_context(tc.tile_pool(name="psS", bufs=1, space="PSUM"))

    # constants / biases (per-partition layouts)
    invD = singles.tile([128, 128], BF16)
    nc.vector.memset(invD, 1.0 / D)  # 2^-10, exact in bf16
    bin_t = singles.tile([128, FT], F32)
    nc.sync.dma_start(bin_t, bin_b.rearrange("(f p) -> p f", p=128))
    bb_t = singles.tile([128, NB, FT], F32)
    nc.sync.dma_start(bb_t, bb.rearrange("i (f p) -> p i f", p=128))
    bout_t = singles.tile([128, FT3], F32)
    nc.sync.dma_start(bout_t, bout.rearrange("(f p) -> p f", p=128))

    # resident state: y.T and t = x.T / normalized y.T, feature-major bf16
    y_t = ypool.tile([128, KT, T], BF16)
    t_t = tpool.tile([128, KT, T], BF16)

    # ---- stats helpers (per token half: 2 chunks of 512) ----
    def make_stats(half):
        sa = [[ysqpool.tile([128, 512], BF16, name="sp", tag=f"sp{j}_{c}")
               for c in range(2)] for j in range(4)]
        qa = [[ysqpool.tile([128, 512], BF16, name="qp", tag=f"qp{j}_{c}")
               for c in range(2)] for j in range(4)]
        return half, sa, qa

    def stats_pair(stats, f):
        """Level-1 kt-partials for feature pair (f-1, f) on this half's two
        chunks: sa = y+y' on Pool; qa = y^2+y'^2 in one custom DVE op."""
        half, sa, qa = stats
        j = f // 2
        for ci in range(2):
            cs = bass.ts(2 * half + ci, 512)
            nc.gpsimd.tensor_tensor(sa[j][ci], y_t[:, f - 1, cs],
                                    y_t[:, f, cs], OP.add)
            nc.vector._custom_dve(SQSQ, out=qa[j][ci],
                                  in0=y_t[:, f - 1, cs], in1=y_t[:, f, cs])

    def stats_reduce_var(stats, ci, sq):
        """Chunk ci: partition-reduce with 4-step PSUM accumulation over the
        raw pair partials (PE, 1/D weights -> mu, E[y^2]; no DVE lvl2/3
        trees), musq on Act, var = (qd + eps) - musq on DVE."""
        half, sa, qa = stats
        mu, qd = sq[:, 0, :], sq[:, 1, :]
        nc.tensor.matmul(mu, invD, sa[0][ci], start=True, stop=True)
        nc.tensor.matmul(qd, invD, qa[0][ci], start=True, stop=True)
        musq = statpool.tile([128, 512], F32, name="musq", tag=f"musq{ci}")
        nc.scalar.square(musq, mu)
        var = statpool.tile([128, 512], F32, name="var", tag=f"var{ci}")
        nc.vector.scalar_tensor_tensor(var, qd, EPS, musq,
                                       OP.add, OP.subtract)
        return var

    def stats_rstd(var, half, ci, rstd, seed):
        """Chunk ci: rstd = rsqrt(var) by Newton iteration, entirely on
        DVE (no Act Sqrt -> no ACT_TABLE_LOADs). Iter 1 is affine in var
        (seed is a per-layer constant); the last iter writes bf16 rstd."""
        c = 2 * half + ci
        r1 = statpool.tile([128, 512], F32, name="rs", tag=f"rs{ci}a")
        nc.vector.tensor_scalar(r1, var, -0.5 * seed**3, 1.5 * seed,
                                OP.mult, OP.add)
        r2 = statpool.tile([128, 512], F32, name="rs", tag=f"rs{ci}b")
        nc.vector._custom_dve(RSQRT_NR, out=r2, in0=r1, in1=var,
                              s0=1.5, s1=0.5)
        nc.vector._custom_dve(RSQRT_NR, out=rstd[:, bass.ts(c, 512)],
                              in0=r2, in1=var, s0=1.5, s1=0.5)

    def norm_part(rstd, half, k0, k1):
        """t = y * rstd for kt in [k0, k1) on this half."""
        hs = bass.ts(half, 1024)
        for kt in range(k0, k1):
            nc.vector.tensor_tensor(t_t[:, kt, hs], y_t[:, kt, hs],
                                    rstd[:, hs], OP.mult)

    def pending_step(pending, f, varbox):
        """Spread the deferred finish of the previous half's stats over this
        half's f slots. Returns pending (None once fully consumed)."""
        if pending is None:
            return None
        st, rstd, half, seed = pending
        if f == 2:
            # lvl2+lvl3 combines (4 partials -> 1) on DVE, emitted here so
            # the waits on Pool's last sa partials never head-block the DVE
            # queue at the half boundary. ci=0 first: its S/Q matmul fires
            # one f-slot earlier.
            _, sa, qa = st
            for ci in range(2):
                nc.vector.tensor_tensor(sa[0][ci], sa[0][ci], sa[1][ci],
                                        OP.add)
                nc.vector.tensor_tensor(sa[2][ci], sa[2][ci], sa[3][ci],
                                        OP.add)
                nc.vector.tensor_tensor(sa[0][ci], sa[0][ci], sa[2][ci],
                                        OP.add)
                nc.vector.tensor_tensor(qa[0][ci], qa[0][ci], qa[1][ci],
                                        OP.add)
                nc.vector.tensor_tensor(qa[2][ci], qa[2][ci], qa[3][ci],
                                        OP.add)
                nc.vector.tensor_tensor(qa[0][ci], qa[0][ci], qa[2][ci],
                                        OP.add)
        elif f == 3:
            sq0 = psS.tile([128, 2, 512], F32, name="SQ", tag="SQ")
            varbox.append(stats_reduce_var(st, 0, sq0))
        elif f == 4:
            sq1 = psG.tile([128, 2, 512], F32, name="G", tag="G")
            varbox.append(stats_reduce_var(st, 1, sq1))
            stats_rstd(varbox.pop(0), half, 0, rstd, seed)
        elif f == 5:
            stats_rstd(varbox.pop(0), half, 1, rstd, seed)
            norm_part(rstd, half, 0, 4)
        elif f == 6:
            norm_part(rstd, half, 4, 8)
            return None
        return pending

    def mm_half(w_tile, f, half, rhs):
        """8-kt accumulation for fout tile f on this half's two chunks."""
        G = psG.tile([128, 2, 512], F32, name="G", tag="G")
        for kt in range(KT):
            lhsT = w_tile[:, kt, bass.ts(f, 128)]
            for ci in range(2):
                nc.tensor.matmul(G[:, ci, :], lhsT,
                                 rhs[:, kt, bass.ts(2 * half + ci, 512)],
                                 start=(kt == 0), stop=(kt == KT - 1))
        return G

    for _rep in range(repeat):
        # ---------------- Phase 1: y.T = (x @ Win.T + bin).T ----------------
        w_in = wpool.tile([128, KT, D], BF16, tag="w")
        nc.sync.dma_start(w_in, wint.rearrange("(kt p) n -> p kt n", p=128))
        for kt in range(KT):
            nc.sync.dma_start(t_t[:, kt, :],
                              xt.rearrange("(kt p) t -> p kt t", p=128)[:, kt, :])

        rstd_next = rstdpool.tile([128, T], BF16, name="rstd", tag="rstd")
        pending = None  # (stats, rstd, half) whose finish+norm is deferred
        varbox = []
        for half in range(NC2):
            st = make_stats(half)
            for f in range(FT):
                G = mm_half(w_in, f, half, t_t)
                nc.scalar.activation(
                    y_t[:, f, bass.ts(half, 1024)],
                    G.rearrange("p a b -> p (a b)"),
                    AF.Identity, bias=bin_t[:, bass.ds(f, 1)])
                if f % 2 == 1:
                    stats_pair(st, f)
                pending = pending_step(pending, f, varbox)
            pending = (st, rstd_next, half, seeds[0])

        # ---------------- Phase 2: residual blocks ----------------
        for i in range(NB):
            wb = wpool.tile([128, KT, D], BF16, tag="w")
            nc.sync.dma_start(wb, wbt[i].rearrange("(kt p) n -> p kt n", p=128))
            if i < NB - 1:
                rstd_next = rstdpool.tile([128, T], BF16, name="rstd", tag="rstd")
            for half in range(NC2):
                st = make_stats(half) if i < NB - 1 else None
                for f in range(FT):
                    G = mm_half(wb, f, half, t_t)
                    g = gpool.tile([128, 1024], BF16, name="g", tag="g")
                    nc.scalar.activation(
                        g, G.rearrange("p a b -> p (a b)"),
                        AF.Gelu, bias=bb_t[:, i, bass.ds(f, 1)])
                    nc.vector.tensor_tensor(y_t[:, f, bass.ts(half, 1024)],
                                            y_t[:, f, bass.ts(half, 1024)],
                                            g, OP.add)
                    if st is not None and f % 2 == 1:
                        stats_pair(st, f)
                    pending = pending_step(pending, f, varbox)
                if st is not None:
                    pending = (st, rstd_next, half, seeds[1 + i])

        # ---------------- Phase 3: out.T = (y @ Wout.T + bout).T ------------
        w3a = wpool.tile([128, KT, D], BF16, tag="w")
        nc.sync.dma_start(w3a, woutt[:, 0:D].rearrange("(kt p) n -> p kt n", p=128))
        w3b = wpool.tile([128, KT, D], BF16, tag="w")
        nc.sync.dma_start(w3b, woutt[:, D:NOUT].rearrange("(kt p) n -> p kt n", p=128))
        for half_w, w3 in ((0, w3a), (1, w3b)):
            for f in range(FT):
                fg = half_w * FT + f
                ost = ostpool.tile([128, T], F32, name="ost", tag="ost")
                for half in range(NC2):
                    G = mm_half(w3, f, half, y_t)
                    nc.scalar.activation(
                        ost[:, bass.ts(half, 1024)],
                        G.rearrange("p a b -> p (a b)"),
                        AF.Identity, bias=bout_t[:, bass.ds(fg, 1)])
                nc.gpsimd.dma_start(out_t[bass.ts(fg, 128), :], ost)


_CACHED_NC = {}


def _estimate_seeds(x, Win, bin_b, Wb, bb):
    """Per-stats-layer rsqrt(mean token variance) — Newton seeds for the
    on-chip rstd iteration. Estimated from a strided token subsample with
    the same LN/gelu math as the reference (seed needs only ~20% accuracy;
    4 Newton iterations converge from up to ~50% off)."""
    try:
        from scipy.special import erf

        def gelu(h):
            return 0.5 * h * (1.0 + erf(h / np.sqrt(2.0)))
    except ImportError:
        def gelu(h):
            return 0.5 * h * (1.0 + np.tanh(0.7978845608 * (h + 0.044715 * h**3)))

    xs = np.asarray(x[:: max(1, x.shape[0] // 512)], np.float32)
    Win = np.asarray(Win, np.float32)
    Wb = np.asarray(Wb, np.float32)
    bb = np.asarray(bb, np.float32)
    y = xs @ Win.T + np.asarray(bin_b, np.float32)
    seeds = []
    for i in range(NB):
        var = y.var(axis=1) + EPS
        seeds.append(float(1.0 / np.sqrt(var.mean())))
        mu = y.mean(axis=1, keepdims=True)
        t = (y - mu) / np.sqrt(var[:, None])
        h = t @ Wb[i].T + bb[i]
        y = y + gelu(h)
    return tuple(round(s, 4) for s in seeds)


def _prep_inputs(x, Win, bin_b, Wb, bb, Wout, bout_b):
    import ml_dtypes
    x = np.asarray(x, np.float32)
    Win = np.asarray(Win, np.float32)
    Wb = np.asarray(Wb, np.float32)
    Wout = np.asarray(Wout, np.float32)
    # fold LN mean-subtraction into the inner-block weights:
    # W' = W - outer(rowsum(W), ones)/D  so  W' @ (y*rstd) == ((y-mu)*rstd) @ W.T
    Wbp = Wb - Wb.sum(axis=2, keepdims=True) / D
    xt = np.ascontiguousarray(x.T).astype(ml_dtypes.bfloat16)
    wint = np.ascontiguousarray(Win.T).astype(ml_dtypes.bfloat16)
    wbt = np.ascontiguousarray(Wbp.transpose(0, 2, 1)).astype(ml_dtypes.bfloat16)
    woutt = np.ascontiguousarray(Wout.T).astype(ml_dtypes.bfloat16)
    return (xt, wint, wbt, woutt, np.asarray(bin_b, np.float32),
            np.asarray(bb, np.float32), np.asarray(bout_b, np.float32))


def make_in_maps(x, Win, bin_b, Wb, bb, Wout, bout_b):
    xt, wint, wbt, woutt, bin_arr, bb_arr, bout_arr = _prep_inputs(
        x, Win, bin_b, Wb, bb, Wout, bout_b)
    in_maps = []
    for c in range(N_CORES):
        in_maps.append({
            "xt": np.ascontiguousarray(xt[:, c * T:(c + 1) * T]),
            "wint": wint, "wbt": wbt, "woutt": woutt,
            "bin_b": bin_arr, "bb": bb_arr, "bout": bout_arr,
        })
    return in_maps


def kernel(x, Win, bin_b, Wb, bb, Wout, bout_b):
    seeds = _estimate_seeds(x, Win, bin_b, Wb, bb)
    nc = _CACHED_NC.get(seeds)
    if nc is None:
        nc = _CACHED_NC[seeds] = build_nc(seeds=seeds)
    in_maps = make_in_maps(x, Win, bin_b, Wb, bb, Wout, bout_b)
    res = bass_utils.run_bass_kernel_spmd(nc, in_maps, list(range(N_CORES)))
    return np.concatenate(
        [np.ascontiguousarray(res.results[c]["out_t"].T) for c in range(N_CORES)],
        axis=0)
